# revision 3
# baseline (speedup 1.0000x reference)
"""Trainium2 Bass kernel for nn_Head: per-batch single-head attention.

reference (per batch b of 1024):
  k = x1 @ Wk.T; q = x2 @ Wq.T; v = x2 @ Wv.T      (E=1001 -> H=64, S=100)
  out = softmax((q @ k.T)/8) @ v                    ([100, 64] per batch)

Strategy: pure data-parallel over 8 cores (128 batches each). Host-side we
pre-transpose + bf16-cast the activations so every device DMA is contiguous
and the PE contracts over e on the partition dim:
  x1t/x2t: [1024(e,pad), 12800(b*s)] bf16
Projections run with the (padded, zero-filled) weight tiles stationary and
x.T streaming; attention per batch uses the "flipped" orientation
  WT = k_T.T @ q_T  -> exp on ACT -> E=[j,i] (unnormalized softmax.T)
  vnat = PE-transpose(v_T); O/sumcol from lhsT=E in one rhs=[vnat|ones] MM
  normalize with DVE reciprocal + per-partition tensor_scalar mult.
"""
import sys
import types
import numpy as np

sys.path.insert(0, "/opt/trn_rl_repo")

import ml_dtypes
from contextlib import ExitStack

import concourse.bass as bass
import concourse.tile as tile
from concourse import mybir
from concourse.bass_utils import run_bass_kernel_spmd

BF16 = mybir.dt.bfloat16
F32 = mybir.dt.float32
NPBF16 = ml_dtypes.bfloat16

NCORES = 8
B, S, E, H = 1024, 100, 1001, 64
BSH = B // NCORES          # 128 batches per core
BS = BSH * S               # 12800 rows per core
EP = 1024                  # e padded to 8*128
NET = 8                    # e-tiles of 128
SGB = 16                   # batches per super-group
NSG = BSH // SGB           # 8 super-groups
SGC = SGB * S              # 1600 cols per super-group
CH = 400                   # psum chunk cols
NCH = SGC // CH            # 4 chunks per sg


def _split_waits(nc, max_waits=1):
    """walrus TRN2 codegen allows only one sem-wait command per instruction;
    move excess waits onto preceding NOPs on the same engine."""
    n_new = 0
    for fn in nc.m.functions:
        for bb in fn.blocks:
            new_insts = []
            for inst in bb.instructions:
                si = inst.sync_info
                if si is not None and len(si.on_wait) > max_waits:
                    waits = list(si.on_wait)
                    while len(waits) > max_waits:
                        chunk, waits = waits[:max_waits], waits[max_waits:]
                        nop = mybir.InstNoOp(
                            name=f"waitsplit_{n_new}",
                            engine=inst.engine,
                            ins=[],
                            outs=[],
                            sync_info=mybir.SyncInfo(on_wait=chunk, on_update=[]),
                            bass_nofuse=True,
                        )
                        n_new += 1
                        new_insts.append(nop)
                    si.on_wait = waits
                new_insts.append(inst)
            bb.instructions[:] = new_insts


def build_kernel():
    nc = bass.Bass(target_bir_lowering=False)
    x1t = nc.declare_dram_parameter("x1t", [EP, BS], BF16, isOutput=False)
    x2t = nc.declare_dram_parameter("x2t", [EP, BS], BF16, isOutput=False)
    wqv = nc.declare_dram_parameter("wqv", [EP, 128], BF16, isOutput=False)
    wk = nc.declare_dram_parameter("wk", [EP, H], BF16, isOutput=False)
    identv = nc.declare_dram_parameter("identv", [128, H], BF16, isOutput=False)
    ones = nc.declare_dram_parameter("ones", [128, 1], BF16, isOutput=False)
    out = nc.declare_dram_parameter("out", [BS, H], F32, isOutput=True)

    with tile.TileContext(nc) as tc, ExitStack() as ctx:
        wpool = ctx.enter_context(tc.tile_pool(name="w", bufs=1))
        xpool = ctx.enter_context(tc.tile_pool(name="x", bufs=2))
        qkv = ctx.enter_context(tc.tile_pool(name="qkv", bufs=2))
        att = ctx.enter_context(tc.tile_pool(name="att", bufs=3))
        fin = ctx.enter_context(tc.tile_pool(name="fin", bufs=2))
        pproj = ctx.enter_context(tc.tile_pool(name="pp", bufs=2, space="PSUM"))
        patt = ctx.enter_context(tc.tile_pool(name="pa", bufs=1, space="PSUM"))
        pao = ctx.enter_context(tc.tile_pool(name="pao", bufs=1, space="PSUM"))

        # persistent constants
        wqv_sb = wpool.tile([128, NET * 128], BF16, tag="wqv")
        nc.sync.dma_start(
            wqv_sb[:],
            bass.AP(wqv, 0, [[128, 128], [128 * 128, NET], [1, 128]]),
        )
        wk_sb = wpool.tile([128, NET * H], BF16, tag="wk")
        nc.sync.dma_start(
            wk_sb[:], bass.AP(wk, 0, [[H, 128], [128 * H, NET], [1, H]])
        )
        iv_sb = wpool.tile([128, H], BF16, tag="iv")
        nc.sync.dma_start(iv_sb[:], identv[:, :])
        ones_sb = wpool.tile([128, 1], BF16, tag="ones")
        nc.sync.dma_start(ones_sb[:], ones[:, :])
        # rhs of the AV matmul: [vnat | ones]; col H set once here, vnat
        # cols rewritten per batch
        rhs_sb = wpool.tile([128, H + 1], BF16, tag="rhs")
        nc.vector.tensor_copy(rhs_sb[:, H : H + 1], ones_sb[:, 0:1])

        for sg in range(NSG):
            # ---- load x.T slices for this super-group (all 8 e-tiles) ----
            x1_sb = xpool.tile([128, NET * SGC], BF16, tag="x1")
            x2_sb = xpool.tile([128, NET * SGC], BF16, tag="x2")
            for xsb, xdr in ((x1_sb, x1t), (x2_sb, x2t)):
                nc.sync.dma_start(
                    xsb[:],
                    bass.AP(
                        xdr,
                        sg * SGC,
                        [[BS, 128], [128 * BS, NET], [1, SGC]],
                    ),
                )

            # ---- projections: q_T/v_T (x2) and k_T (x1), e contracted ----
            qv_sb = qkv.tile([128, SGC], BF16, tag="qv")
            k_sb = qkv.tile([H, SGC], BF16, tag="k")
            for c in range(NCH):
                pa = pproj.tile([128, CH], F32, tag="pa")
                pb = pproj.tile([H, CH], F32, tag="pb")
                for et in range(NET):
                    nc.tensor.matmul(
                        pa[:],
                        lhsT=wqv_sb[:, et * 128 : (et + 1) * 128],
                        rhs=x2_sb[:, et * SGC + c * CH : et * SGC + (c + 1) * CH],
                        start=(et == 0),
                        stop=(et == NET - 1),
                    )
                for et in range(NET):
                    nc.tensor.matmul(
                        pb[:],
                        lhsT=wk_sb[:, et * H : (et + 1) * H],
                        rhs=x1_sb[:, et * SGC + c * CH : et * SGC + (c + 1) * CH],
                        start=(et == 0),
                        stop=(et == NET - 1),
                    )
                nc.vector.tensor_copy(qv_sb[:, c * CH : (c + 1) * CH], pa[:])
                nc.vector.tensor_copy(k_sb[:, c * CH : (c + 1) * CH], pb[:])

            # ---- attention, one batch at a time ----
            fin_sb = fin.tile([S, SGB * H], F32, tag="fin")
            for b in range(SGB):
                c0 = b * S
                q_T = qv_sb[0:H, c0 : c0 + S]
                v_T = qv_sb[H : 2 * H, c0 : c0 + S]
                k_T = k_sb[0:H, c0 : c0 + S]

                wt_ps = patt.tile([S, S], F32, tag="wt")
                nc.tensor.matmul(wt_ps[:], lhsT=k_T, rhs=q_T)

                e_sb = att.tile([S, S], BF16, tag="e")
                nc.scalar.activation(
                    e_sb[:], wt_ps[:],
                    mybir.ActivationFunctionType.Exp, scale=0.125,
                )

                vn_ps = patt.tile([S, H], BF16, tag="vn")
                nc.tensor.matmul(
                    vn_ps[:], lhsT=v_T, rhs=iv_sb[H:128, :], is_transpose=True
                )
                nc.vector.tensor_copy(rhs_sb[0:S, 0:H], vn_ps[:])

                o_ps = pao.tile([S, H + 1], F32, tag="o")
                nc.tensor.matmul(o_ps[:], lhsT=e_sb[:], rhs=rhs_sb[0:S, :])

                r_sb = att.tile([S, 1], F32, tag="r")
                nc.vector.reciprocal(r_sb[:], o_ps[:, H : H + 1])
                nc.vector.tensor_scalar_mul(
                    fin_sb[:, b * H : (b + 1) * H], o_ps[:, 0:H], r_sb[:]
                )

            nc.sync.dma_start(
                bass.AP(out, sg * SGC * H, [[H, S], [S * H, SGB], [1, H]]),
                fin_sb[:],
            )

    _split_waits(nc)
    return nc


_NC_CACHE = [None]


def kernel(x1, x2, Wk, Wq, Wv):
    if _NC_CACHE[0] is None:
        _NC_CACHE[0] = build_kernel()
    nc = _NC_CACHE[0]

    wqv_np = np.zeros((EP, 128), dtype=NPBF16)
    wqv_np[:E, :H] = Wq.T.astype(NPBF16)
    wqv_np[:E, H:] = Wv.T.astype(NPBF16)
    wk_np = np.zeros((EP, H), dtype=NPBF16)
    wk_np[:E] = Wk.T.astype(NPBF16)
    iv_np = np.zeros((128, H), dtype=NPBF16)
    iv_np[H:, :] = np.eye(H, dtype=NPBF16)
    ones_np = np.ones((128, 1), dtype=NPBF16)

    in_maps = []
    for c in range(NCORES):
        m = {"wqv": wqv_np, "wk": wk_np, "identv": iv_np, "ones": ones_np}
        for name, x in (("x1t", x1), ("x2t", x2)):
            xs = x[c * BSH : (c + 1) * BSH].reshape(BS, E).astype(NPBF16)
            xt = np.zeros((EP, BS), dtype=NPBF16)
            xt[:E] = xs.T
            m[name] = xt
        in_maps.append(m)

    global _last_in_maps
    _last_in_maps = in_maps
    res = run_bass_kernel_spmd(nc, in_maps, list(range(NCORES)))
    return np.concatenate(
        [res.results[c]["out"].reshape(BSH, S, H) for c in range(NCORES)], axis=0
    )


# revision 6
# speedup vs baseline: 1.1874x; 1.1874x over previous
"""Trainium2 Bass kernel for nn_Head: per-batch single-head attention.

reference (per batch b of 1024):
  k = x1 @ Wk.T; q = x2 @ Wq.T; v = x2 @ Wv.T      (E=1001 -> H=64, S=100)
  out = softmax((q @ k.T)/8) @ v                    ([100, 64] per batch)

Strategy: pure data-parallel over 8 cores (128 batches each). Host-side we
pre-transpose + bf16-cast the activations so every device DMA is contiguous
and the PE contracts over e on the partition dim:
  x1t/x2t: [1024(e,pad), 12800(b*s)] bf16
Projections run with the (padded, zero-filled) weight tiles stationary and
x.T streaming; attention per batch uses the "flipped" orientation
  WT = k_T.T @ q_T  -> exp on ACT -> E=[j,i] (unnormalized softmax.T)
  vnat = PE-transpose(v_T); O/sumcol from lhsT=E in one rhs=[vnat|ones] MM
  normalize with DVE reciprocal + per-partition tensor_scalar mult.
"""
import sys
import types
import numpy as np

sys.path.insert(0, "/opt/trn_rl_repo")

import ml_dtypes
from contextlib import ExitStack

import concourse.bass as bass
import concourse.tile as tile
from concourse import mybir
from concourse.bass_utils import run_bass_kernel_spmd

BF16 = mybir.dt.bfloat16
F32 = mybir.dt.float32
NPBF16 = ml_dtypes.bfloat16

NCORES = 8
B, S, E, H = 1024, 100, 1001, 64
BSH = B // NCORES          # 128 batches per core
BS = BSH * S               # 12800 rows per core
EP = 1024                  # e padded to 8*128
NET = 8                    # e-tiles of 128
SGB = 16                   # batches per super-group
NSG = BSH // SGB           # 8 super-groups
SGC = SGB * S              # 1600 cols per super-group
CH = 400                   # psum chunk cols
NCH = SGC // CH            # 4 chunks per sg


def _split_waits(nc, max_waits=1):
    """walrus TRN2 codegen allows only one sem-wait command per instruction;
    move excess waits onto preceding NOPs on the same engine."""
    n_new = 0
    for fn in nc.m.functions:
        for bb in fn.blocks:
            new_insts = []
            for inst in bb.instructions:
                si = inst.sync_info
                if si is not None and len(si.on_wait) > max_waits:
                    waits = list(si.on_wait)
                    while len(waits) > max_waits:
                        chunk, waits = waits[:max_waits], waits[max_waits:]
                        nop = mybir.InstNoOp(
                            name=f"waitsplit_{n_new}",
                            engine=inst.engine,
                            ins=[],
                            outs=[],
                            sync_info=mybir.SyncInfo(on_wait=chunk, on_update=[]),
                            bass_nofuse=True,
                        )
                        n_new += 1
                        new_insts.append(nop)
                    si.on_wait = waits
                new_insts.append(inst)
            bb.instructions[:] = new_insts


def build_kernel():
    nc = bass.Bass(target_bir_lowering=False)
    x1t = nc.declare_dram_parameter("x1t", [EP, BS], BF16, isOutput=False)
    x2t = nc.declare_dram_parameter("x2t", [EP, BS], BF16, isOutput=False)
    wqv = nc.declare_dram_parameter("wqv", [EP, 128], BF16, isOutput=False)
    wk = nc.declare_dram_parameter("wk", [EP, H], BF16, isOutput=False)
    identv = nc.declare_dram_parameter("identv", [128, H], BF16, isOutput=False)
    ones = nc.declare_dram_parameter("ones", [128, 1], BF16, isOutput=False)
    out = nc.declare_dram_parameter("out", [BS, H], F32, isOutput=True)

    with tile.TileContext(nc) as tc, ExitStack() as ctx:
        wpool = ctx.enter_context(tc.tile_pool(name="w", bufs=1))
        xpool = ctx.enter_context(tc.tile_pool(name="x", bufs=2))
        qkv = ctx.enter_context(tc.tile_pool(name="qkv", bufs=2))
        att = ctx.enter_context(tc.tile_pool(name="att", bufs=3))
        fin = ctx.enter_context(tc.tile_pool(name="fin", bufs=2))
        pproj = ctx.enter_context(tc.tile_pool(name="pp", bufs=1, space="PSUM"))
        patt = ctx.enter_context(tc.tile_pool(name="pa", bufs=2, space="PSUM"))
        pao = ctx.enter_context(tc.tile_pool(name="pao", bufs=2, space="PSUM"))

        # persistent constants
        wqv_sb = wpool.tile([128, NET * 128], BF16, tag="wqv")
        nc.sync.dma_start(
            wqv_sb[:],
            bass.AP(wqv, 0, [[128, 128], [128 * 128, NET], [1, 128]]),
        )
        wk_sb = wpool.tile([128, NET * H], BF16, tag="wk")
        nc.sync.dma_start(
            wk_sb[:], bass.AP(wk, 0, [[H, 128], [128 * H, NET], [1, H]])
        )
        iv_sb = wpool.tile([128, H], BF16, tag="iv")
        nc.sync.dma_start(iv_sb[:], identv[:, :])
        ones_sb = wpool.tile([128, 1], BF16, tag="ones")
        nc.sync.dma_start(ones_sb[:], ones[:, :])


        for sg in range(NSG):
            # ---- load x.T slices for this super-group (all 8 e-tiles) ----
            x1_sb = xpool.tile([128, NET * SGC], BF16, tag="x1")
            x2_sb = xpool.tile([128, NET * SGC], BF16, tag="x2")
            for xsb, xdr in ((x1_sb, x1t), (x2_sb, x2t)):
                nc.sync.dma_start(
                    xsb[:],
                    bass.AP(
                        xdr,
                        sg * SGC,
                        [[BS, 128], [128 * BS, NET], [1, SGC]],
                    ),
                )

            # ---- projections: q_T/v_T (x2) and k_T (x1), e contracted ----
            qv_sb = qkv.tile([128, SGC], BF16, tag="qv")
            k_sb = qkv.tile([H, SGC], BF16, tag="k")
            for c in range(NCH):
                pa = pproj.tile([128, CH], F32, tag="pa")
                pb = pproj.tile([H, CH], F32, tag="pb")
                for et in range(NET):
                    nc.tensor.matmul(
                        pa[:],
                        lhsT=wqv_sb[:, et * 128 : (et + 1) * 128],
                        rhs=x2_sb[:, et * SGC + c * CH : et * SGC + (c + 1) * CH],
                        start=(et == 0),
                        stop=(et == NET - 1),
                    )
                for et in range(NET):
                    nc.tensor.matmul(
                        pb[:],
                        lhsT=wk_sb[:, et * H : (et + 1) * H],
                        rhs=x1_sb[:, et * SGC + c * CH : et * SGC + (c + 1) * CH],
                        start=(et == 0),
                        stop=(et == NET - 1),
                    )
                nc.vector.tensor_copy(qv_sb[:, c * CH : (c + 1) * CH], pa[:])
                nc.vector.tensor_copy(k_sb[:, c * CH : (c + 1) * CH], pb[:])

            # ---- attention, one batch at a time ----
            fin_sb = fin.tile([S, SGB * H], F32, tag="fin")
            for b in range(SGB):
                c0 = b * S
                q_T = qv_sb[0:H, c0 : c0 + S]
                v_T = qv_sb[H : 2 * H, c0 : c0 + S]
                k_T = k_sb[0:H, c0 : c0 + S]

                wt_ps = patt.tile([S, S], F32, tag="wt")
                nc.tensor.matmul(wt_ps[:], lhsT=k_T, rhs=q_T)

                e_sb = att.tile([S, S], BF16, tag="e")
                nc.scalar.activation(
                    e_sb[:], wt_ps[:],
                    mybir.ActivationFunctionType.Exp, scale=0.125,
                )

                vn_ps = patt.tile([S, H], BF16, tag="vn")
                nc.tensor.matmul(
                    vn_ps[:], lhsT=v_T, rhs=iv_sb[H:128, :], is_transpose=True
                )
                rhs_t = att.tile([S, H + 1], BF16, tag="rhs")
                nc.vector.tensor_copy(rhs_t[:, 0:H], vn_ps[:])
                nc.vector.tensor_copy(rhs_t[:, H : H + 1], ones_sb[0:S, 0:1])

                o_ps = pao.tile([S, H + 1], F32, tag="o")
                nc.tensor.matmul(o_ps[:], lhsT=e_sb[:], rhs=rhs_t[:])

                r_sb = att.tile([S, 1], F32, tag="r")
                nc.vector.reciprocal(r_sb[:], o_ps[:, H : H + 1])
                nc.vector.tensor_scalar_mul(
                    fin_sb[:, b * H : (b + 1) * H], o_ps[:, 0:H], r_sb[:]
                )

            nc.sync.dma_start(
                bass.AP(out, sg * SGC * H, [[H, S], [S * H, SGB], [1, H]]),
                fin_sb[:],
            )

    _split_waits(nc)
    return nc


_NC_CACHE = [None]


def kernel(x1, x2, Wk, Wq, Wv):
    if _NC_CACHE[0] is None:
        _NC_CACHE[0] = build_kernel()
    nc = _NC_CACHE[0]

    wqv_np = np.zeros((EP, 128), dtype=NPBF16)
    wqv_np[:E, :H] = Wq.T.astype(NPBF16)
    wqv_np[:E, H:] = Wv.T.astype(NPBF16)
    wk_np = np.zeros((EP, H), dtype=NPBF16)
    wk_np[:E] = Wk.T.astype(NPBF16)
    iv_np = np.zeros((128, H), dtype=NPBF16)
    iv_np[H:, :] = np.eye(H, dtype=NPBF16)
    ones_np = np.ones((128, 1), dtype=NPBF16)

    in_maps = []
    for c in range(NCORES):
        m = {"wqv": wqv_np, "wk": wk_np, "identv": iv_np, "ones": ones_np}
        for name, x in (("x1t", x1), ("x2t", x2)):
            xs = x[c * BSH : (c + 1) * BSH].reshape(BS, E).astype(NPBF16)
            xt = np.zeros((EP, BS), dtype=NPBF16)
            xt[:E] = xs.T
            m[name] = xt
        in_maps.append(m)

    global _last_in_maps
    _last_in_maps = in_maps
    res = run_bass_kernel_spmd(nc, in_maps, list(range(NCORES)))
    return np.concatenate(
        [res.results[c]["out"].reshape(BSH, S, H) for c in range(NCORES)], axis=0
    )


# revision 8
# speedup vs baseline: 1.1981x; 1.0090x over previous
"""Trainium2 Bass kernel for nn_Head: per-batch single-head attention.

reference (per batch b of 1024):
  k = x1 @ Wk.T; q = x2 @ Wq.T; v = x2 @ Wv.T      (E=1001 -> H=64, S=100)
  out = softmax((q @ k.T)/8) @ v                    ([100, 64] per batch)

Strategy: pure data-parallel over 8 cores (128 batches each). Host-side we
pre-transpose + bf16-cast the activations so every device DMA is contiguous
and the PE contracts over e on the partition dim:
  x1t/x2t: [1024(e,pad), 12800(b*s)] bf16
Projections run with the (padded, zero-filled) weight tiles stationary and
x.T streaming; attention per batch uses the "flipped" orientation
  WT = k_T.T @ q_T  -> exp on ACT -> E=[j,i] (unnormalized softmax.T)
  vnat = PE-transpose(v_T); O/sumcol from lhsT=E in one rhs=[vnat|ones] MM
  normalize with DVE reciprocal + per-partition tensor_scalar mult.
"""
import sys
import types
import numpy as np

sys.path.insert(0, "/opt/trn_rl_repo")

import ml_dtypes
from contextlib import ExitStack

import concourse.bass as bass
import concourse.tile as tile
from concourse import mybir
from concourse.bass_utils import run_bass_kernel_spmd

BF16 = mybir.dt.bfloat16
F32 = mybir.dt.float32
NPBF16 = ml_dtypes.bfloat16

NCORES = 8
B, S, E, H = 1024, 100, 1001, 64
BSH = B // NCORES          # 128 batches per core
BS = BSH * S               # 12800 rows per core
EP = 1024                  # e padded to 8*128
NET = 8                    # e-tiles of 128
SGB = 16                   # batches per super-group
NSG = BSH // SGB           # 8 super-groups
SGC = SGB * S              # 1600 cols per super-group
CH = 400                   # psum chunk cols
NCH = SGC // CH            # 4 chunks per sg


def _split_waits(nc, max_waits=1):
    """walrus TRN2 codegen allows only one sem-wait command per instruction;
    move excess waits onto preceding NOPs on the same engine."""
    n_new = 0
    for fn in nc.m.functions:
        for bb in fn.blocks:
            new_insts = []
            for inst in bb.instructions:
                si = inst.sync_info
                if si is not None and len(si.on_wait) > max_waits:
                    waits = list(si.on_wait)
                    while len(waits) > max_waits:
                        chunk, waits = waits[:max_waits], waits[max_waits:]
                        nop = mybir.InstNoOp(
                            name=f"waitsplit_{n_new}",
                            engine=inst.engine,
                            ins=[],
                            outs=[],
                            sync_info=mybir.SyncInfo(on_wait=chunk, on_update=[]),
                            bass_nofuse=True,
                        )
                        n_new += 1
                        new_insts.append(nop)
                    si.on_wait = waits
                new_insts.append(inst)
            bb.instructions[:] = new_insts


def build_kernel():
    nc = bass.Bass(target_bir_lowering=False)
    x1t = nc.declare_dram_parameter("x1t", [EP, BS], BF16, isOutput=False)
    x2t = nc.declare_dram_parameter("x2t", [EP, BS], BF16, isOutput=False)
    wqv = nc.declare_dram_parameter("wqv", [EP, 128], BF16, isOutput=False)
    wk = nc.declare_dram_parameter("wk", [EP, H], BF16, isOutput=False)
    identv = nc.declare_dram_parameter("identv", [128, H], BF16, isOutput=False)
    ones = nc.declare_dram_parameter("ones", [128, 1], BF16, isOutput=False)
    out = nc.declare_dram_parameter("out", [BS, H], F32, isOutput=True)

    with tile.TileContext(nc) as tc, ExitStack() as ctx:
        wpool = ctx.enter_context(tc.tile_pool(name="w", bufs=1))
        xpool = ctx.enter_context(tc.tile_pool(name="x", bufs=2))
        qkv = ctx.enter_context(tc.tile_pool(name="qkv", bufs=2))
        att = ctx.enter_context(tc.tile_pool(name="att", bufs=3))
        fin = ctx.enter_context(tc.tile_pool(name="fin", bufs=2))
        pproj = ctx.enter_context(tc.tile_pool(name="pp", bufs=1, space="PSUM"))
        patt = ctx.enter_context(tc.tile_pool(name="pa", bufs=2, space="PSUM"))
        pao = ctx.enter_context(tc.tile_pool(name="pao", bufs=2, space="PSUM"))

        # persistent constants
        wqv_sb = wpool.tile([128, NET * 128], BF16, tag="wqv")
        nc.sync.dma_start(
            wqv_sb[:],
            bass.AP(wqv, 0, [[128, 128], [128 * 128, NET], [1, 128]]),
        )
        wk_sb = wpool.tile([128, NET * H], BF16, tag="wk")
        nc.sync.dma_start(
            wk_sb[:], bass.AP(wk, 0, [[H, 128], [128 * H, NET], [1, H]])
        )
        iv_sb = wpool.tile([128, H], BF16, tag="iv")
        nc.sync.dma_start(iv_sb[:], identv[:, :])
        ones_sb = wpool.tile([128, 1], BF16, tag="ones")
        nc.sync.dma_start(ones_sb[:], ones[:, :])


        for sg in range(NSG):
            # ---- load x.T slices for this super-group (all 8 e-tiles) ----
            x1_sb = xpool.tile([128, NET * SGC], BF16, tag="x1")
            x2_sb = xpool.tile([128, NET * SGC], BF16, tag="x2")
            for xsb, xdr in ((x1_sb, x1t), (x2_sb, x2t)):
                nc.sync.dma_start(
                    xsb[:],
                    bass.AP(
                        xdr,
                        sg * SGC,
                        [[BS, 128], [128 * BS, NET], [1, SGC]],
                    ),
                )

            # ---- projections: q_T/v_T (x2) and k_T (x1), e contracted ----
            qv_sb = qkv.tile([128, SGC], BF16, tag="qv")
            k_sb = qkv.tile([H, SGC], BF16, tag="k")
            for c in range(NCH):
                pa = pproj.tile([128, CH], F32, tag="pa")
                pb = pproj.tile([H, CH], F32, tag="pb")
                for et in range(NET):
                    nc.tensor.matmul(
                        pa[:],
                        lhsT=wqv_sb[:, et * 128 : (et + 1) * 128],
                        rhs=x2_sb[:, et * SGC + c * CH : et * SGC + (c + 1) * CH],
                        start=(et == 0),
                        stop=(et == NET - 1),
                    )
                for et in range(NET):
                    nc.tensor.matmul(
                        pb[:],
                        lhsT=wk_sb[:, et * H : (et + 1) * H],
                        rhs=x1_sb[:, et * SGC + c * CH : et * SGC + (c + 1) * CH],
                        start=(et == 0),
                        stop=(et == NET - 1),
                    )
                nc.vector.tensor_copy(qv_sb[:, c * CH : (c + 1) * CH], pa[:])
                nc.vector.tensor_copy(k_sb[:, c * CH : (c + 1) * CH], pb[:])

            # ---- attention: logits+exp batched 4 batches per ACT op ----
            fin_sb = fin.tile([S, SGB * H], F32, tag="fin")
            for b0 in range(0, SGB, 4):
                wt_ps = patt.tile([S, 4 * S], F32, tag="wt")
                for i in range(4):
                    c0 = (b0 + i) * S
                    nc.tensor.matmul(
                        wt_ps[:, i * S : (i + 1) * S],
                        lhsT=k_sb[0:H, c0 : c0 + S],
                        rhs=qv_sb[0:H, c0 : c0 + S],
                    )
                e4_sb = att.tile([S, 4 * S], BF16, tag="e")
                nc.scalar.activation(
                    e4_sb[:], wt_ps[:],
                    mybir.ActivationFunctionType.Exp, scale=0.125,
                )
                for i in range(4):
                    b = b0 + i
                    c0 = b * S
                    v_T = qv_sb[H : 2 * H, c0 : c0 + S]
                    e_sb = e4_sb[:, i * S : (i + 1) * S]

                    vn_ps = patt.tile([S, H], BF16, tag="vn")
                    nc.tensor.matmul(
                        vn_ps[:], lhsT=v_T, rhs=iv_sb[H:128, :], is_transpose=True
                    )
                    rhs_t = att.tile([S, H + 1], BF16, tag="rhs")
                    nc.vector.tensor_copy(rhs_t[:, 0:H], vn_ps[:])
                    nc.vector.tensor_copy(rhs_t[:, H : H + 1], ones_sb[0:S, 0:1])

                    o_ps = pao.tile([S, H + 1], F32, tag="o")
                    nc.tensor.matmul(o_ps[:], lhsT=e_sb, rhs=rhs_t[:])

                    r_sb = att.tile([S, 1], F32, tag="r")
                    nc.vector.reciprocal(r_sb[:], o_ps[:, H : H + 1])
                    nc.vector.tensor_scalar_mul(
                        fin_sb[:, b * H : (b + 1) * H], o_ps[:, 0:H], r_sb[:]
                    )

            nc.sync.dma_start(
                bass.AP(out, sg * SGC * H, [[H, S], [S * H, SGB], [1, H]]),
                fin_sb[:],
            )

    _split_waits(nc)
    return nc


_NC_CACHE = [None]


def kernel(x1, x2, Wk, Wq, Wv):
    if _NC_CACHE[0] is None:
        _NC_CACHE[0] = build_kernel()
    nc = _NC_CACHE[0]

    wqv_np = np.zeros((EP, 128), dtype=NPBF16)
    wqv_np[:E, :H] = Wq.T.astype(NPBF16)
    wqv_np[:E, H:] = Wv.T.astype(NPBF16)
    wk_np = np.zeros((EP, H), dtype=NPBF16)
    wk_np[:E] = Wk.T.astype(NPBF16)
    iv_np = np.zeros((128, H), dtype=NPBF16)
    iv_np[H:, :] = np.eye(H, dtype=NPBF16)
    ones_np = np.ones((128, 1), dtype=NPBF16)

    in_maps = []
    for c in range(NCORES):
        m = {"wqv": wqv_np, "wk": wk_np, "identv": iv_np, "ones": ones_np}
        for name, x in (("x1t", x1), ("x2t", x2)):
            xs = x[c * BSH : (c + 1) * BSH].reshape(BS, E).astype(NPBF16)
            xt = np.zeros((EP, BS), dtype=NPBF16)
            xt[:E] = xs.T
            m[name] = xt
        in_maps.append(m)

    global _last_in_maps
    _last_in_maps = in_maps
    res = run_bass_kernel_spmd(nc, in_maps, list(range(NCORES)))
    return np.concatenate(
        [res.results[c]["out"].reshape(BSH, S, H) for c in range(NCORES)], axis=0
    )
